# revision 4
# baseline (speedup 1.0000x reference)
"""Trainium2 Bass kernel for causal self-attention with RoPE.

Problem: B=2, T=2048, C=1024, H=16 heads, hd=64, fp32 in/out, causal, rotary.

Sharding: 8 cores = 2 batches x 4 head-groups. Core c handles batch c//4 and
heads [4*(c%4), 4*(c%4)+4). Each core computes its heads' Q/K/V projections,
RoPE, causal attention, and a partial output projection over its 256 input
channels; the host sums the 4 partial projections per batch and adds bp.

Schedule (per core): everything is emitted as ~512-col units so each PSUM
tile is one bank and the PE stream never starves:
  A: QK projections+rope for t<1024, V' tiles s<1024.
  B: QK h1 / V' 8..15 units interleaved into the w0/w1 attention rounds
     (2-head round-robin per pair) as PE filler while exp runs on ACT.
  C: w2/w3 attention in 4-head round-robin with output-projection units
     (t<1024) as filler.
  D: tail projections for t in [1536,2048).
All matmul operands are fp16 (1 cycle/row on the PE, and 2-byte dtypes make
DVE ops eligible for the 2x/4x fast modes). Output is fp16; the host
accumulates the four partial projections in fp32 and adds the bias.
"""

import time
from collections import deque
from contextlib import ExitStack

import numpy as np

import concourse.bass as bass
import concourse.tile as tile
from concourse import bacc, library_config, mybir
from concourse.bass_utils import run_bass_kernel_spmd

F32 = mybir.dt.float32
F16 = mybir.dt.float16

T = 2048
C = 1024
HD = 64
NCORES = 8
NEG = -1e10

AF = mybir.ActivationFunctionType
ALU = mybir.AluOpType

LAST_EXEC_NS = None
LAST_RESULTS = None


def build_nc():
    nc = bacc.Bacc("TRN2", target_bir_lowering=False, debug=False)

    xT = nc.dram_tensor("xT", [C + 1, T], F16, kind="ExternalInput").ap()
    wqT = nc.dram_tensor("wqT", [C, 256], F16, kind="ExternalInput").ap()
    wkT = nc.dram_tensor("wkT", [C, 256], F16, kind="ExternalInput").ap()
    wvT = nc.dram_tensor("wvT", [C + 128, 256], F16, kind="ExternalInput").ap()
    wpT = nc.dram_tensor("wpT", [256, C], F16, kind="ExternalInput").ap()
    bqk = nc.dram_tensor("bqk", [128, 4], F32, kind="ExternalInput").ap()
    cc_d = nc.dram_tensor("cc", [128, T], F16, kind="ExternalInput").ap()
    ss_d = nc.dram_tensor("ss", [128, T], F16, kind="ExternalInput").ap()
    tri_d = nc.dram_tensor("tri", [128, 128], F32, kind="ExternalInput").ap()
    out_d = nc.dram_tensor("out", [T, C], F16, kind="ExternalOutput").ap()

    with tile.TileContext(nc) as tc, ExitStack() as ctx:
        consts = ctx.enter_context(tc.tile_pool(name="consts", bufs=1))
        nc.gpsimd.load_library(library_config.attn)

        cc_sb = consts.tile([128, T], F16)
        ss_sb = consts.tile([128, T], F16)
        tri_sb = consts.tile([128, 128], F32)
        bqk_sb = consts.tile([128, 4], F32)
        x1 = consts.tile([1, T], F16)

        # rotated Q^T / K^T: [pair][half] tiles (Q pairs 0-1, K pairs 2-3)
        qkt = [[consts.tile([128, 1024], F16, name=f"qkt{p}_{h}")
                for h in range(2)] for p in range(4)]
        # V' tiles per s-tile: [128 s, 4*65] (64 v-cols + ones col per head)
        vp = [consts.tile([128, 4 * 65], F16, name=f"vp{i}") for i in range(16)]
        vview = [v.rearrange("p (h d) -> p h d", d=65) for v in vp]
        # normalized attention outputs, per pair per 512-col window
        usc = [[consts.tile([128, 512], F16, name=f"usc{p}_{w}")
                for w in range(4)] for p in range(2)]
        wp_sb = [consts.tile([128, C], F16, name=f"wp{p}") for p in range(2)]
        xts = [consts.tile([128, T], F16, name=f"xt{j}") for j in range(8)]
        wq_sb = [consts.tile([128, 256], F16, name=f"wq{j}") for j in range(8)]
        wk_sb = [consts.tile([128, 256], F16, name=f"wk{j}") for j in range(8)]
        wv_sb = [consts.tile([128, 256], F16, name=f"wv{j}") for j in range(9)]
        wv1 = wv_sb[8]

        h0, h1 = slice(0, 1024), slice(1024, 2048)

        # ---------------- input DMA, priority order ----------------
        nc.sync.dma_start(wq_sb[0][:], wqT[0:128, :])
        nc.sync.dma_start(xts[0][:, h0], xT[0:128, h0])
        nc.sync.dma_start(cc_sb[:, h0], cc_d[:, h0])
        nc.sync.dma_start(ss_sb[:, h0], ss_d[:, h0])
        nc.sync.dma_start(bqk_sb[:], bqk[:])
        for j in range(1, 8):
            nc.sync.dma_start(wq_sb[j][:], wqT[128 * j:128 * (j + 1), :])
            nc.sync.dma_start(xts[j][:, h0], xT[128 * j:128 * (j + 1), h0])
        nc.sync.dma_start(tri_sb[:], tri_d[:])
        for j in range(8):
            nc.sync.dma_start(wk_sb[j][:], wkT[128 * j:128 * (j + 1), :])
        for j in range(9):
            nc.sync.dma_start(wv_sb[j][:], wvT[128 * j:128 * (j + 1), :])
        for j in range(8):
            nc.sync.dma_start(xts[j][:, h1], xT[128 * j:128 * (j + 1), h1])
        nc.sync.dma_start(cc_sb[:, h1], cc_d[:, h1])
        nc.sync.dma_start(ss_sb[:, h1], ss_d[:, h1])
        for p in range(2):
            nc.sync.dma_start(wp_sb[p][:], wpT[128 * p:128 * (p + 1), :])

        nc.gpsimd.memset(x1[:], 1.0)
        for i in range(16):
            nc.gpsimd.memset(vview[i][:, :, 64], 1.0)

        # persistent pools
        rp = ctx.enter_context(tc.tile_pool(name="rope", bufs=2))
        epl = ctx.enter_context(tc.tile_pool(name="epool", bufs=10))
        zrp = ctx.enter_context(tc.tile_pool(name="zrpool", bufs=4))
        rzbp = ctx.enter_context(tc.tile_pool(name="rzbpool", bufs=4))
        ost = ctx.enter_context(tc.tile_pool(name="ostage", bufs=4))
        yzp = ctx.enter_context(
            tc.tile_pool(name="yzpsum", bufs=4, space="PSUM"))
        sps = ctx.enter_context(
            tc.tile_pool(name="spsum", bufs=2, space="PSUM"))

        # ---------------- unit definitions ----------------
        def qk_unit(mmp, wsb, ci, half, tg):
            """One 512-col projection+rope unit for Q/K chunk ci."""
            csl = slice(1024 * half + 512 * tg, 1024 * half + 512 * tg + 512)
            wsl = slice(512 * tg, 512 * tg + 512)  # within-half columns
            ps = mmp.tile([128, 512], F32, tag="mm",
                          name=f"qk{wsb is wk_sb}_{ci}_{half}_{tg}")
            for j in range(8):
                nc.tensor.matmul(ps[:], wsb[j][:, 128 * ci:128 * (ci + 1)],
                                 xts[j][:, csl], start=(j == 0), stop=(j == 7))
            bcol = (2 if wsb is wk_sb else 0) + ci
            pair = (2 if wsb is wk_sb else 0) + ci
            bias = bqk_sb[:, bcol:bcol + 1]
            p1 = rp.tile([128, 512], F16, tag="p1")
            p2 = rp.tile([128, 512], F16, tag="p2")
            p2s = rp.tile([128, 512], F16, tag="p2s")
            nc.vector.scalar_tensor_tensor(
                out=p1[:], in0=ps[:], scalar=bias, in1=cc_sb[:, csl],
                op0=ALU.add, op1=ALU.mult)
            nc.vector.scalar_tensor_tensor(
                out=p2[:], in0=ps[:], scalar=bias, in1=ss_sb[:, csl],
                op0=ALU.add, op1=ALU.mult)
            for r in range(4):
                src = slice(32 * (r ^ 1), 32 * (r ^ 1) + 32)
                dst = slice(32 * r, 32 * r + 32)
                nc.sync.dma_start(p2s[dst, :], p2[src, :])
            nc.vector.tensor_add(qkt[pair][half][:, wsl], p1[:], p2s[:])

        def v_unit(mmp, i):
            """V' s-tile i: vraw = x_i^T @ Wv (+bias), copy into vp[i]."""
            ps = mmp.tile([128, 512], F32, tag="mm", name=f"v{i}")
            tsl = slice(128 * i, 128 * (i + 1))
            for j in range(8):
                nc.tensor.matmul(ps[:, 0:256], xts[j][:, tsl], wv_sb[j][:],
                                 start=(j == 0), stop=False)
            nc.tensor.matmul(ps[:, 0:256], x1[:, tsl], wv1[0:1, :],
                             start=False, stop=True)
            nc.scalar.activation(vview[i][:, :, 0:64], ps[:, 0:256], AF.Copy)

        yz_live = {}

        def score_unit(pr, hs, w, i):
            """Scores for head (pr,hs), window w, s-tile i; exp to an e-tile."""
            h = 2 * pr + hs
            rows = slice(64 * hs, 64 * (hs + 1))
            sub0 = max(0, 128 * i - 512 * w)
            kt = qkt[2 + pr][i // 8]
            qt = qkt[pr][w // 2]
            qsl = slice((512 * w) % 1024 + sub0, (512 * w) % 1024 + 512)
            s_ps = sps.tile([128, 512], F32, tag="s", name=f"s{h}_{w}_{i}")
            nc.tensor.matmul(
                s_ps[:, sub0:512],
                kt[rows, 128 * (i % 8):128 * (i % 8) + 128],
                qt[rows, qsl], start=True, stop=True)
            if i >= 4 * w:
                nc.vector.tensor_add(
                    s_ps[:, sub0:sub0 + 128], s_ps[:, sub0:sub0 + 128],
                    tri_sb[:])
            et = epl.tile([128, 512], F16, tag="e", name=f"e{h}_{w}_{i}")
            nc.scalar.activation(et[:, sub0:512], s_ps[:, sub0:512],
                                 AF.Exp, scale=0.125)
            return et, sub0

        def attv_unit(pr, hs, w, i, et, sub0, ni):
            h = 2 * pr + hs
            if i == 0:
                yz_live[h] = yzp.tile([65, 512], F32, tag="yz",
                                      name=f"yz{h}_{w}")
            yz = yz_live[h]
            nc.tensor.matmul(yz[:, sub0:512], vp[i][:, 65 * h:65 * (h + 1)],
                             et[:, sub0:512], start=(i == 0),
                             stop=(i == ni - 1))

        def norm_unit(pr, hs, w):
            h = 2 * pr + hs
            yz = yz_live[h]
            zrow = zrp.tile([1, 512], F32, tag="zrow", name=f"zr{h}_{w}")
            nc.vector.tensor_copy(zrow[:], yz[64:65, :])
            rzr = zrp.tile([1, 512], F32, tag="rzr", name=f"rr{h}_{w}")
            nc.vector.reciprocal_approx_fast(rzr[:], zrow[:])
            rzb = rzbp.tile([64, 512], F32, tag="rzb", name=f"rb{h}_{w}")
            nc.gpsimd.partition_broadcast(rzb[:], rzr[:])
            nc.vector.tensor_mul(
                usc[pr][w][64 * hs:64 * (hs + 1), :], yz[0:64, :], rzb[:])

        def proj_unit(ppp, tch, cg):
            w = tch // 4
            tsl = slice(128 * (tch % 4), 128 * (tch % 4) + 128)
            csl = slice(512 * cg, 512 * (cg + 1))
            ps = ppp.tile([128, 512], F32, tag="op", name=f"op{tch}_{cg}")
            for pq in range(2):
                nc.tensor.matmul(ps[:], usc[pq][w][:, tsl],
                                 wp_sb[pq][:, csl],
                                 start=(pq == 0), stop=(pq == 1))
            st = ost.tile([128, 512], F16, tag="ost", name=f"st{tch}_{cg}")
            nc.vector.tensor_copy(st[:], ps[:])
            nc.sync.dma_start(out_d[128 * tch:128 * tch + 128, csl], st[:])

        # ---------------- A + B blocks (mm psum pool scope) ----------------
        with tc.tile_pool(name="mmpsum", bufs=2, space="PSUM") as mmp:
            # A: QK h0 + V' 0..7
            for ci in range(2):
                for tg in range(2):
                    qk_unit(mmp, wq_sb, ci, 0, tg)
            for ci in range(2):
                for tg in range(2):
                    qk_unit(mmp, wk_sb, ci, 0, tg)
            for i in range(8):
                v_unit(mmp, i)

            # B: w0/w1 attention (2-head RR per pair) + h1/V filler units
            yunits = deque()
            for ci in range(2):
                for tg in range(2):
                    yunits.append(lambda ci=ci, tg=tg:
                                  qk_unit(mmp, wq_sb, ci, 1, tg))
                    yunits.append(lambda ci=ci, tg=tg:
                                  qk_unit(mmp, wk_sb, ci, 1, tg))
            for i in range(8, 16):
                yunits.append(lambda i=i: v_unit(mmp, i))

            nyb = len(yunits)
            rounds_b = 2 * (4 + 8)
            emitted = 0
            ridx = 0
            for w in range(2):
                ni = 4 * w + 4
                for pr in range(2):
                    for i in range(ni):
                        ea = score_unit(pr, 0, w, i)
                        eb = score_unit(pr, 1, w, i)
                        while emitted < (ridx + 1) * nyb // rounds_b:
                            yunits.popleft()()
                            emitted += 1
                        attv_unit(pr, 0, w, i, *ea, ni)
                        attv_unit(pr, 1, w, i, *eb, ni)
                        ridx += 1
                    norm_unit(pr, 0, w)
                    norm_unit(pr, 1, w)

        # ---------------- C + D blocks (proj psum pool scope) ----------------
        with tc.tile_pool(name="oppsum", bufs=2, space="PSUM") as ppp:
            yunits = deque()
            for tch in range(8):
                for cg in range(2):
                    yunits.append(lambda tch=tch, cg=cg:
                                  proj_unit(ppp, tch, cg))
            ny8 = len(yunits)

            for w in range(2, 4):
                ni = 4 * w + 4
                if w == 3:
                    # w2 usc is ready: queue its projection units
                    for tch in range(8, 12):
                        for cg in range(2):
                            yunits.append(lambda tch=tch, cg=cg:
                                          proj_unit(ppp, tch, cg))
                nyc = len(yunits)
                emitted = 0
                for i in range(ni):
                    e0 = score_unit(0, 0, w, i)
                    e1 = score_unit(0, 1, w, i)
                    if emitted < (i + 1) * nyc // ni and yunits:
                        yunits.popleft()()
                        emitted += 1
                    e2 = score_unit(1, 0, w, i)
                    e3 = score_unit(1, 1, w, i)
                    attv_unit(0, 0, w, i, *e0, ni)
                    attv_unit(0, 1, w, i, *e1, ni)
                    if emitted < (i + 1) * nyc // ni and yunits:
                        yunits.popleft()()
                        emitted += 1
                    attv_unit(1, 0, w, i, *e2, ni)
                    attv_unit(1, 1, w, i, *e3, ni)
                while yunits and emitted < nyc:
                    yunits.popleft()()
                    emitted += 1
                for pr in range(2):
                    for hs in range(2):
                        norm_unit(pr, hs, w)

            # D: tail projections for w3
            for tch in range(12, 16):
                for cg in range(2):
                    proj_unit(ppp, tch, cg)

    nc.compile()
    return nc


_NC_CACHE = {}


def _get_nc():
    if "nc" not in _NC_CACHE:
        _NC_CACHE["nc"] = build_nc()
    return _NC_CACHE["nc"]


def make_in_map(core, x, Wq, bq, Wk, bk, Wv, bv, Wp, bp, rope_cache):
    b = core // 4
    hbase = (core % 4) * 4

    xTa = np.empty((C + 1, T), np.float16)
    xTa[:C] = np.asarray(x[b], np.float32).T
    xTa[C] = 1.0

    # packed channel order for Q/K: per pair p, heads (hbase+2p, hbase+2p+1),
    # rows [hA_even(32) | hA_odd(32) | hB_even(32) | hB_odd(32)]
    perm = []
    for p in range(2):
        for hh in range(2):
            h = hbase + 2 * p + hh
            perm += [h * HD + 2 * m for m in range(32)]
            perm += [h * HD + 2 * m + 1 for m in range(32)]
    perm = np.asarray(perm)

    wqTa = np.ascontiguousarray(
        np.asarray(Wq, np.float32)[perm, :].T).astype(np.float16)
    wkTa = np.ascontiguousarray(
        np.asarray(Wk, np.float32)[perm, :].T).astype(np.float16)

    chs = np.arange(hbase * HD, hbase * HD + 256)
    wvTa = np.zeros((C + 128, 256), np.float16)
    wvTa[:C] = np.asarray(Wv, np.float32)[chs, :].T
    wvTa[C] = np.asarray(bv, np.float32)[chs]
    wpTa = np.ascontiguousarray(
        np.asarray(Wp, np.float32)[:, chs].T).astype(np.float16)

    bqp = np.asarray(bq, np.float32)[perm].reshape(2, 128).T
    bkp = np.asarray(bk, np.float32)[perm].reshape(2, 128).T
    bqk_a = np.concatenate([bqp, bkp], axis=1)  # [128, 4]

    rc = np.asarray(rope_cache, np.float32)  # [T, 32, 2]
    r = np.arange(128)
    m = r % 32
    sign = np.where((r % 64) < 32, 1.0, -1.0).astype(np.float32)
    cc_a = np.ascontiguousarray(rc[:, m, 0].T).astype(np.float16)
    ss_a = np.ascontiguousarray(
        (rc[:, m, 1].T * sign[:, None])).astype(np.float16)

    sl, tl = np.arange(128)[:, None], np.arange(128)[None, :]
    tri_a = np.where(tl >= sl, 0.0, NEG).astype(np.float32)

    return dict(xT=xTa, wqT=wqTa, wkT=wkTa, wvT=wvTa, wpT=wpTa,
                bqk=bqk_a, cc=cc_a, ss=ss_a, tri=tri_a)


def kernel(x, Wq, bq, Wk, bk, Wv, bv, Wp, bp, rope_cache):
    global LAST_EXEC_NS, LAST_RESULTS
    args = (x, Wq, bq, Wk, bk, Wv, bv, Wp, bp, rope_cache)
    nc = _get_nc()
    in_maps = [make_in_map(c, *args) for c in range(NCORES)]
    r = None
    for attempt in range(4):
        try:
            r = run_bass_kernel_spmd(nc, in_maps, list(range(NCORES)))
            break
        except Exception:
            # transient NRT exec-unit errors recover on re-dispatch
            if attempt == 3:
                raise
            time.sleep(5.0 * (attempt + 1))
    LAST_EXEC_NS = r.exec_time_ns
    LAST_RESULTS = r
    out = np.zeros((2, T, C), np.float32)
    for core in range(NCORES):
        out[core // 4] += np.asarray(r.results[core]["out"], np.float32)
    out += np.asarray(bp, np.float32)[None, None, :]
    return out


# revision 9
# speedup vs baseline: 1.1054x; 1.1054x over previous
"""Trainium2 Bass kernel for causal self-attention with RoPE.

Problem: B=2, T=2048, C=1024, H=16 heads, hd=64, fp32 in/out, causal, rotary.

Sharding: 8 cores = 2 batches x 4 head-groups. Core c handles batch c//4 and
heads [4*(c%4), 4*(c%4)+4). Each core computes its heads' Q/K/V projections,
RoPE, causal attention, and a partial output projection over its 256 input
channels; the host sums the 4 partial projections per batch and adds bp.

Key scheduling ideas (everything is ~512-col units, one PSUM bank each, with
manual bank assignment over the 8 banks):
  A: QK projections+rope for t<1024 and V' tiles s<1024.
  B: w0/w1 attention (2-head round-robin per pair) with the QK-h1 / V' 8..15
     units injected between rounds as PE filler.
  C: w2/w3 attention in 4-head round-robin with output-projection units
     (t<1024) as filler.
  D: tail projections for t in [1536,2048).
attV lags scores by 2 rounds so exp (ACT) and the normalization chains are
off the PE critical path. The RoPE half-rotation is a DVE stream_shuffle
(channel pairs packed 16 apart inside each 32-partition quadrant). All matmul
operands are fp16; output is fp16 and the host accumulates in fp32.
"""

import time
from collections import deque
from contextlib import ExitStack

import numpy as np

import concourse.bass as bass
import concourse.tile as tile
from concourse import bacc, library_config, mybir
from concourse.bass_utils import run_bass_kernel_spmd

F32 = mybir.dt.float32
F16 = mybir.dt.float16

T = 2048
C = 1024
HD = 64
NCORES = 8
NEG = -1e10
LAG = 2
SWAP_MASK = list(range(16, 32)) + list(range(16))

AF = mybir.ActivationFunctionType
ALU = mybir.AluOpType

LAST_EXEC_NS = None
LAST_RESULTS = None


def build_nc():
    nc = bacc.Bacc("TRN2", target_bir_lowering=False, debug=False)

    xT = nc.dram_tensor("xT", [C + 1, T], F16, kind="ExternalInput").ap()
    wqT = nc.dram_tensor("wqT", [C, 256], F16, kind="ExternalInput").ap()
    wkT = nc.dram_tensor("wkT", [C, 256], F16, kind="ExternalInput").ap()
    wvT = nc.dram_tensor("wvT", [C + 128, 256], F16, kind="ExternalInput").ap()
    wpT = nc.dram_tensor("wpT", [256, C], F16, kind="ExternalInput").ap()
    bqk = nc.dram_tensor("bqk", [128, 4], F32, kind="ExternalInput").ap()
    cc_d = nc.dram_tensor("cc", [128, T], F16, kind="ExternalInput").ap()
    ss_d = nc.dram_tensor("ss", [128, T], F16, kind="ExternalInput").ap()
    tri_d = nc.dram_tensor("tri", [128, 128], F32, kind="ExternalInput").ap()
    out_d = nc.dram_tensor("out", [T, C], F16, kind="ExternalOutput").ap()

    with tile.TileContext(nc) as tc, ExitStack() as ctx:
        consts = ctx.enter_context(tc.tile_pool(name="consts", bufs=1))
        nc.gpsimd.load_library(library_config.attn)

        cc_sb = consts.tile([128, T], F16)
        ss_sb = consts.tile([128, T], F16)
        tri_sb = consts.tile([128, 128], F32)
        bqk_sb = consts.tile([128, 4], F32)
        x1 = consts.tile([1, T], F16)

        # rotated Q^T / K^T: [pair][half] tiles (Q pairs 0-1, K pairs 2-3)
        qkt = [[consts.tile([128, 1024], F16, name=f"qkt{p}_{h}")
                for h in range(2)] for p in range(4)]
        vp = [consts.tile([128, 4 * 65], F16, name=f"vp{i}") for i in range(16)]
        vview = [v.rearrange("p (h d) -> p h d", d=65) for v in vp]
        usc = [[consts.tile([128, 512], F16, name=f"usc{p}_{w}")
                for w in range(4)] for p in range(2)]
        wp_sb = [consts.tile([128, C], F16, name=f"wp{p}") for p in range(2)]
        xts = [consts.tile([128, T], F16, name=f"xt{j}") for j in range(8)]
        wq_sb = [consts.tile([128, 256], F16, name=f"wq{j}") for j in range(8)]
        wk_sb = [consts.tile([128, 256], F16, name=f"wk{j}") for j in range(8)]
        wv_sb = [consts.tile([128, 256], F16, name=f"wv{j}") for j in range(9)]
        wv1 = wv_sb[8]

        h0, h1 = slice(0, 1024), slice(1024, 2048)

        # ------------- input DMA: priority order, spread over queues -------
        nc.sync.dma_start(cc_sb[:, 0:512], cc_d[:, 0:512])
        nc.sync.dma_start(ss_sb[:, 0:512], ss_d[:, 0:512])
        for j in range(8):
            nc.sync.dma_start(wq_sb[j][:], wqT[128 * j:128 * (j + 1), :])
            nc.sync.dma_start(xts[j][:, h0], xT[128 * j:128 * (j + 1), h0])
        nc.sync.dma_start(bqk_sb[:], bqk[:])
        nc.sync.dma_start(cc_sb[:, 512:1024], cc_d[:, 512:1024])
        nc.sync.dma_start(ss_sb[:, 512:1024], ss_d[:, 512:1024])
        for j in range(8):
            nc.sync.dma_start(wk_sb[j][:], wkT[128 * j:128 * (j + 1), :])
            nc.sync.dma_start(xts[j][:, h1], xT[128 * j:128 * (j + 1), h1])
        for j in range(9):
            nc.sync.dma_start(wv_sb[j][:], wvT[128 * j:128 * (j + 1), :])
        nc.sync.dma_start(tri_sb[:], tri_d[:])
        for q in range(2, 4):
            nc.sync.dma_start(cc_sb[:, 512 * q:512 * (q + 1)],
                              cc_d[:, 512 * q:512 * (q + 1)])
            nc.sync.dma_start(ss_sb[:, 512 * q:512 * (q + 1)],
                              ss_d[:, 512 * q:512 * (q + 1)])
        for p in range(2):
            nc.sync.dma_start(wp_sb[p][:], wpT[128 * p:128 * (p + 1), :])

        nc.gpsimd.memset(x1[:], 1.0)
        for i in range(16):
            nc.gpsimd.memset(vview[i][:, :, 64], 1.0)

        # persistent SBUF pools
        rp = ctx.enter_context(tc.tile_pool(name="rope", bufs=2))
        epl = ctx.enter_context(tc.tile_pool(name="epool", bufs=16))
        zrp = ctx.enter_context(tc.tile_pool(name="zrpool", bufs=4))
        rzbp = ctx.enter_context(tc.tile_pool(name="rzbpool", bufs=4))
        ost = ctx.enter_context(tc.tile_pool(name="ostage", bufs=4))
        # one PSUM pool, manual bank assignment via tags b0..b7
        pb = ctx.enter_context(tc.tile_pool(name="pbank", bufs=1,
                                            space="PSUM"))

        def bank(k, name):
            return pb.tile([128, 512], F32, tag=f"b{k}", name=name)

        mm_ctr = [0]
        s_ctr = [0]

        # ---------------- unit definitions ----------------
        def qk_unit(wsb, ci, half, tg):
            """One 512-col projection+rope unit for Q/K chunk ci."""
            csl = slice(1024 * half + 512 * tg, 1024 * half + 512 * tg + 512)
            wsl = slice(512 * tg, 512 * tg + 512)
            isk = wsb is wk_sb
            ps = bank(mm_ctr[0] % 2, f"qk{isk}_{ci}_{half}_{tg}")
            mm_ctr[0] += 1
            for j in range(8):
                nc.tensor.matmul(ps[:], wsb[j][:, 128 * ci:128 * (ci + 1)],
                                 xts[j][:, csl], start=(j == 0), stop=(j == 7))
            bcol = (2 if isk else 0) + ci
            pair = (2 if isk else 0) + ci
            bias = bqk_sb[:, bcol:bcol + 1]
            p1 = rp.tile([128, 512], F16, tag="p1")
            p2 = rp.tile([128, 512], F16, tag="p2")
            p2s = rp.tile([128, 512], F16, tag="p2s")
            nc.vector.scalar_tensor_tensor(
                out=p1[:], in0=ps[:], scalar=bias, in1=cc_sb[:, csl],
                op0=ALU.add, op1=ALU.mult)
            nc.vector.scalar_tensor_tensor(
                out=p2[:], in0=ps[:], scalar=bias, in1=ss_sb[:, csl],
                op0=ALU.add, op1=ALU.mult)
            nc.vector.stream_shuffle(p2s[:], p2[:], SWAP_MASK)
            nc.vector.tensor_add(qkt[pair][half][:, wsl], p1[:], p2s[:])

        def v_unit(i):
            """V' s-tile i: vraw = x_i^T @ Wv (+bias), copy into vp[i]."""
            ps = bank(mm_ctr[0] % 2, f"v{i}")
            mm_ctr[0] += 1
            tsl = slice(128 * i, 128 * (i + 1))
            for j in range(8):
                nc.tensor.matmul(ps[:, 0:256], xts[j][:, tsl], wv_sb[j][:],
                                 start=(j == 0), stop=False)
            nc.tensor.matmul(ps[:, 0:256], x1[:, tsl], wv1[0:1, :],
                             start=False, stop=True)
            nc.scalar.activation(vview[i][:, :, 0:64], ps[:, 0:256], AF.Copy)

        yz_live = {}

        def score_unit(pr, hs, w, i, nbanks, bank0):
            """Scores for head (pr,hs), window w, s-tile i; exp to an e-tile."""
            h = 2 * pr + hs
            rows = slice(64 * hs, 64 * (hs + 1))
            sub0 = max(0, 128 * i - 512 * w)
            kt = qkt[2 + pr][i // 8]
            qt = qkt[pr][w // 2]
            qsl = slice((512 * w) % 1024 + sub0, (512 * w) % 1024 + 512)
            s_ps = bank(bank0 + s_ctr[0] % nbanks, f"s{h}_{w}_{i}")
            s_ctr[0] += 1
            nc.tensor.matmul(
                s_ps[:, sub0:512],
                kt[rows, 128 * (i % 8):128 * (i % 8) + 128],
                qt[rows, qsl], start=True, stop=True)
            if i >= 4 * w:
                nc.vector.tensor_add(
                    s_ps[:, sub0:sub0 + 128], s_ps[:, sub0:sub0 + 128],
                    tri_sb[:])
            et = epl.tile([128, 512], F16, tag="e", name=f"e{h}_{w}_{i}")
            nc.scalar.activation(et[:, sub0:512], s_ps[:, sub0:512],
                                 AF.Exp, scale=0.125)
            return et, sub0

        def attv_unit(pr, hs, w, i, et, sub0, ni):
            h = 2 * pr + hs
            if i == 0:
                yz_live[h] = bank(4 + h, f"yz{h}_{w}")
            yz = yz_live[h]
            nc.tensor.matmul(yz[0:65, sub0:512],
                             vp[i][:, 65 * h:65 * (h + 1)],
                             et[:, sub0:512], start=(i == 0),
                             stop=(i == ni - 1))

        def norm_unit(pr, hs, w):
            h = 2 * pr + hs
            yz = yz_live[h]
            zrow = zrp.tile([1, 512], F32, tag="zrow", name=f"zr{h}_{w}")
            nc.vector.tensor_copy(zrow[:], yz[64:65, :])
            rzr = zrp.tile([1, 512], F32, tag="rzr", name=f"rr{h}_{w}")
            nc.vector.reciprocal_approx_fast(rzr[:], zrow[:])
            rzb = rzbp.tile([64, 512], F32, tag="rzb", name=f"rb{h}_{w}")
            nc.gpsimd.partition_broadcast(rzb[:], rzr[:])
            nc.vector.tensor_mul(
                usc[pr][w][64 * hs:64 * (hs + 1), :], yz[0:64, :], rzb[:])

        def proj_unit(tch, cg, bk=0):
            w = tch // 4
            tsl = slice(128 * (tch % 4), 128 * (tch % 4) + 128)
            csl = slice(512 * cg, 512 * (cg + 1))
            ps = bank(bk, f"op{tch}_{cg}")
            for pq in range(2):
                nc.tensor.matmul(ps[:], usc[pq][w][:, tsl],
                                 wp_sb[pq][:, csl],
                                 start=(pq == 0), stop=(pq == 1))
            st = ost.tile([128, 512], F16, tag="ost", name=f"st{tch}_{cg}")
            nc.vector.tensor_copy(st[:], ps[:])
            nc.sync.dma_start(out_d[128 * tch:128 * tch + 128, csl], st[:])

        def window_rounds(w, heads, yq, s_nbanks, s_bank0):
            """Attention window w for the given heads, attV lagging LAG
            rounds behind scores; filler units popped between the two."""
            ni = 4 * w + 4
            nr = ni + LAG
            pend = {hh: deque() for hh in heads}
            ny0 = len(yq)
            emitted = 0
            for r in range(nr):
                if r < ni:
                    for hh in heads:
                        pend[hh].append(
                            (r, *score_unit(*hh, w, r, s_nbanks, s_bank0)))
                while yq and emitted < (r + 1) * ny0 // nr:
                    yq.popleft()()
                    emitted += 1
                if r >= LAG:
                    for hh in heads:
                        i, et, sub0 = pend[hh].popleft()
                        attv_unit(*hh, w, i, et, sub0, ni)
            for hh in heads:
                norm_unit(*hh, w)

        # ---------------- A: QK h0 + V' 0..7 ----------------
        aunits = []
        for ci in range(2):
            for tg in range(2):
                aunits.append(lambda ci=ci, tg=tg: qk_unit(wq_sb, ci, 0, tg))
        for ci in range(2):
            for tg in range(2):
                aunits.append(lambda ci=ci, tg=tg: qk_unit(wk_sb, ci, 0, tg))
        for i in range(8):
            aunits.append(lambda i=i: v_unit(i))
        for u in aunits:
            u()

        # ---------------- B: w0/w1 attention + h1/V filler ----------------
        yq = deque()
        for ci in range(2):
            for tg in range(2):
                yq.append(lambda ci=ci, tg=tg: qk_unit(wq_sb, ci, 1, tg))
                yq.append(lambda ci=ci, tg=tg: qk_unit(wk_sb, ci, 1, tg))
        for i in range(8, 16):
            yq.append(lambda i=i: v_unit(i))
        nyb = len(yq)
        # hand out filler roughly evenly across the four pair-windows
        shares = [nyb * 6 // 32, nyb * 12 // 32, nyb * 22 // 32, nyb]
        prev = 0
        for wi, (w, pr) in enumerate([(0, 0), (0, 1), (1, 0), (1, 1)]):
            cnt = shares[wi] - prev
            prev = shares[wi]
            sub = deque(yq.popleft() for _ in range(cnt))
            window_rounds(w, [(pr, 0), (pr, 1)], sub, 2, 2)
            while sub:
                sub.popleft()()

        # ---------------- C: w2/w3 attention + projection filler ----------
        yq = deque()
        for tch in range(8):
            for cg in range(2):
                yq.append(lambda tch=tch, cg=cg: proj_unit(tch, cg))
        window_rounds(2, [(0, 0), (0, 1), (1, 0), (1, 1)], yq, 3, 1)
        while yq:
            yq.popleft()()
        yq = deque()
        for tch in range(8, 12):
            for cg in range(2):
                yq.append(lambda tch=tch, cg=cg: proj_unit(tch, cg))
        window_rounds(3, [(0, 0), (0, 1), (1, 0), (1, 1)], yq, 3, 1)
        while yq:
            yq.popleft()()

        # ---------------- D: tail projections ----------------
        for tch in range(12, 16):
            for cg in range(2):
                proj_unit(tch, cg, bk=(2 * tch + cg) % 2)

    nc.compile()
    return nc


_NC_CACHE = {}


def _get_nc():
    if "nc" not in _NC_CACHE:
        _NC_CACHE["nc"] = build_nc()
    return _NC_CACHE["nc"]


def make_in_map(core, x, Wq, bq, Wk, bk, Wv, bv, Wp, bp, rope_cache):
    b = core // 4
    hbase = (core % 4) * 4

    xTa = np.empty((C + 1, T), np.float16)
    xTa[:C] = np.asarray(x[b], np.float32).T
    xTa[C] = 1.0

    # packed channel order for Q/K: per head, two 32-row quadrants; each
    # quadrant holds [even ch 16q..16q+15 | odd ch 16q..16q+15] so the rope
    # partner swap is lane l -> (l+16)%32 inside every quadrant.
    perm = []
    for p in range(2):
        for hh in range(2):
            h = hbase + 2 * p + hh
            for q in range(2):
                perm += [h * HD + 2 * (16 * q + m) for m in range(16)]
                perm += [h * HD + 2 * (16 * q + m) + 1 for m in range(16)]
    perm = np.asarray(perm)

    wqTa = np.ascontiguousarray(
        np.asarray(Wq, np.float32)[perm, :].T).astype(np.float16)
    wkTa = np.ascontiguousarray(
        np.asarray(Wk, np.float32)[perm, :].T).astype(np.float16)

    chs = np.arange(hbase * HD, hbase * HD + 256)
    wvTa = np.zeros((C + 128, 256), np.float16)
    wvTa[:C] = np.asarray(Wv, np.float32)[chs, :].T
    wvTa[C] = np.asarray(bv, np.float32)[chs]
    wpTa = np.ascontiguousarray(
        np.asarray(Wp, np.float32)[:, chs].T).astype(np.float16)

    bqp = np.asarray(bq, np.float32)[perm].reshape(2, 128).T
    bkp = np.asarray(bk, np.float32)[perm].reshape(2, 128).T
    bqk_a = np.concatenate([bqp, bkp], axis=1)  # [128, 4]

    rc = np.asarray(rope_cache, np.float32)  # [T, 32, 2]
    r = np.arange(128)
    lane = r % 32
    quad = (r // 32) % 2
    m = 16 * quad + (lane % 16)  # rotation pair index per row
    sign = np.where(lane < 16, 1.0, -1.0).astype(np.float32)
    cc_a = np.ascontiguousarray(rc[:, m, 0].T).astype(np.float16)
    ss_a = np.ascontiguousarray(
        (rc[:, m, 1].T * sign[:, None])).astype(np.float16)

    sl, tl = np.arange(128)[:, None], np.arange(128)[None, :]
    tri_a = np.where(tl >= sl, 0.0, NEG).astype(np.float32)

    return dict(xT=xTa, wqT=wqTa, wkT=wkTa, wvT=wvTa, wpT=wpTa,
                bqk=bqk_a, cc=cc_a, ss=ss_a, tri=tri_a)


def kernel(x, Wq, bq, Wk, bk, Wv, bv, Wp, bp, rope_cache):
    global LAST_EXEC_NS, LAST_RESULTS
    args = (x, Wq, bq, Wk, bk, Wv, bv, Wp, bp, rope_cache)
    nc = _get_nc()
    in_maps = [make_in_map(c, *args) for c in range(NCORES)]
    r = None
    for attempt in range(4):
        try:
            r = run_bass_kernel_spmd(nc, in_maps, list(range(NCORES)))
            break
        except Exception:
            # transient NRT exec-unit errors recover on re-dispatch
            if attempt == 3:
                raise
            time.sleep(5.0 * (attempt + 1))
    LAST_EXEC_NS = r.exec_time_ns
    LAST_RESULTS = r
    out = np.zeros((2, T, C), np.float32)
    for core in range(NCORES):
        out[core // 4] += np.asarray(r.results[core]["out"], np.float32)
    out += np.asarray(bp, np.float32)[None, None, :]
    return out
